# revision 25
# baseline (speedup 1.0000x reference)
"""Trainium2 Bass kernel for nn_BlockTransformer (Octo-style block-sparse transformer).

Strategy: data-parallel over batch (B=8 -> 1 element per NeuronCore), weights
replicated. Residual stream kept transposed (D on partitions). Tokens reordered
to [prefix|pad, obs t0..t9, readouts|pad] = 1536 padded tokens; per-key mask
folded into the softmax-exp bias, readout causality via small memsets + one
static multiplier tile.

v4: attention projections (Q/K, V, WO) in fp8e4m3 with DoubleRow perf mode
(256-deep contraction); weights scaled x256 host-side, compensated in the
following DVE op. Softmax's error-washing makes attention fp8 nearly free
accuracy-wise; the FFN is NOT (each fp8 tensor there costs ~1.3e-2 rel err),
so the FFN stays bf16 with weights streamed per (chunk, quarter) and all four
F-quarters accumulated in PSUM -> one residual add per (chunk, d-col). LN sums
feed the PE directly as float32r (no bf16 staging copy). Softmax exp merged
across the head pair (one ACT op over both heads' scores in a paired PSUM
tile). WO/W2 biases enter via a 1-row matmul.
"""
import sys
sys.path.insert(0, "/opt/trn_rl_repo")

import numpy as np
import ml_dtypes

B, HOR, PFX, NO, NR = 8, 10, 16, 128, 8
D, NH, HD, F, L = 768, 12, 64, 3072, 12
TPS = NO + NR
T = PFX + HOR * TPS          # 1376
TP = 1536                    # padded tokens (12 tiles of 128)
TQ = 1488                    # live tokens (prefix tile + 10 obs tiles + 80 readouts)
NT = TP // 128               # 12 token tiles
DC = D // 128                # 6 d-chunks
CHUNKS = [(0, 512), (512, 512), (1024, 464)]   # token chunks (c2 trimmed)
EPS = 1e-6
NEG = -30000.0
WS = 256.0                   # fp8 weight scale
IWS = 1.0 / WS

_CACHE = {}


def _build_nc(n_layers):
    from concourse import bacc
    import concourse.bass as bass
    import concourse.mybir as mybir
    import concourse.tile as tile
    from contextlib import ExitStack

    bf16, f32, f8 = mybir.dt.bfloat16, mybir.dt.float32, mybir.dt.float8e4
    f32r = mybir.dt.float32r
    AF = mybir.ActivationFunctionType
    OP = mybir.AluOpType
    DR = mybir.MatmulPerfMode.DoubleRow

    nc = bacc.Bacc("TRN2", num_devices=8)

    xT_d = nc.dram_tensor("xT", [D, TP], f32, kind="ExternalInput")
    wqk_d = nc.dram_tensor("wqk", [n_layers, D, 2 * D], f8, kind="ExternalInput")
    wv_d = nc.dram_tensor("wv", [n_layers, D, D], f8, kind="ExternalInput")
    bqkvT_d = nc.dram_tensor("bqkvT", [n_layers, 128, 12], f32, kind="ExternalInput")
    wo_d = nc.dram_tensor("wo", [n_layers, D, D], f8, kind="ExternalInput")
    w1_d = nc.dram_tensor("w1", [n_layers, D, F], bf16, kind="ExternalInput")
    b1T_d = nc.dram_tensor("b1T", [n_layers, 128, 24], f32, kind="ExternalInput")
    w2_d = nc.dram_tensor("w2", [n_layers, F, D], bf16, kind="ExternalInput")
    rbias_d = nc.dram_tensor("rbias", [n_layers, 1, 2, D], bf16, kind="ExternalInput")
    lnfsT_d = nc.dram_tensor("lnfsT", [128, DC], f32, kind="ExternalInput")
    lnfbT_d = nc.dram_tensor("lnfbT", [128, DC], f32, kind="ExternalInput")
    maskcol_d = nc.dram_tensor("maskcol", [128, NT], f32, kind="ExternalInput")
    rrmask_d = nc.dram_tensor("rrmask", [128, 128], bf16, kind="ExternalInput")
    outT_d = nc.dram_tensor("outT", [D, TP], f32, kind="ExternalOutput")

    def dchunked(ap):  # [D, N] dram AP -> [128, DC-chunks, N]
        return ap.rearrange("(ko p) n -> p ko n", p=128)

    with tile.TileContext(nc) as tc, ExitStack() as ctx:
        const = ctx.enter_context(tc.tile_pool(name="const", bufs=1))
        persist = ctx.enter_context(tc.tile_pool(name="persist", bufs=1))
        wpool = ctx.enter_context(tc.tile_pool(name="wpool", bufs=4))
        w1pool = ctx.enter_context(tc.tile_pool(name="w1pool", bufs=2))
        w2pool = ctx.enter_context(tc.tile_pool(name="w2pool", bufs=2))
        y2pool = ctx.enter_context(tc.tile_pool(name="y2pool", bufs=1))
        bpool = ctx.enter_context(tc.tile_pool(name="bpool", bufs=2))
        bigpool = ctx.enter_context(tc.tile_pool(name="bigpool", bufs=1))
        hpool = ctx.enter_context(tc.tile_pool(name="hpool", bufs=2))
        qkpool = ctx.enter_context(tc.tile_pool(name="qkpool", bufs=2))
        lnpool = ctx.enter_context(tc.tile_pool(name="lnpool", bufs=1))
        mupool = ctx.enter_context(tc.tile_pool(name="mupool", bufs=2))
        lnbig = ctx.enter_context(tc.tile_pool(name="lnbig", bufs=1))
        recpool = ctx.enter_context(tc.tile_pool(name="recpool", bufs=1))
        psA = ctx.enter_context(tc.tile_pool(name="psA", bufs=3, space="PSUM"))
        psB = ctx.enter_context(tc.tile_pool(name="psB", bufs=2, space="PSUM"))

        def psA_tile():
            return psA.tile([128, 1024], f32, tag="a", name="psa")

        def psB_tile():
            return psB.tile([128, 512], f32, tag="b", name="psb")

        # ---- persistent state ----
        xT = persist.tile([128, DC, TP], f32)         # residual stream (transposed)
        nc.sync.dma_start(xT, dchunked(xT_d[:]))
        vone = persist.tile([128, NT, NH * 128], f8)  # per head: [V_h | ones]
        nc.vector.memset(vone, 1.0)
        yT8 = persist.tile([128, DC, TP], f8)         # LN1 out (shared by QKV)
        nc.vector.memset(yT8[:, :, TQ:TP], 0.0)
        ARENA_OFF = []
        _o = 0
        for kt in range(NT):
            ARENA_OFF.append(_o)
            _o += TQ - (0 if kt == 0 else 128 * kt)
        # two fp8 exp(score) arenas -> both heads of a pair live concurrently
        ptarena2 = persist.tile([128, 2, _o], f8, tag="ptarena2")

        # ---- constants ----
        maskcol = const.tile([128, NT], f32)
        nc.sync.dma_start(maskcol, maskcol_d[:])
        rrm = const.tile([128, 128], bf16)
        nc.sync.dma_start(rrm, rrmask_d[:])
        onesPP = const.tile([128, 128], bf16)
        nc.vector.memset(onesPP, 1.0)
        onerow = const.tile([1, 512], bf16)
        nc.vector.memset(onerow, 1.0)
        epsT = const.tile([128, 1], f32)
        nc.vector.memset(epsT, EPS)
        lnfs = const.tile([128, DC], f32)
        nc.sync.dma_start(lnfs, lnfsT_d[:])
        lnfb = const.tile([128, DC], f32)
        nc.sync.dma_start(lnfb, lnfbT_d[:])

        def ln_stats(c0, n):
            """returns (mu, rstd, both f32 [128, n]) for token chunk [c0, c0+n)."""
            sl = slice(c0, c0 + n)
            xsq = lnbig.tile([128, DC, 512], bf16, tag="xsq")
            nc.scalar.activation(xsq[:, :, :n], xT[:, :, sl], AF.Square)
            xb = lnbig.tile([128, DC, 512], bf16, tag="xb")
            nc.gpsimd.tensor_copy(xb[:, :, :n], xT[:, :, sl])
            ps = psA_tile()
            sums = ps[:, :n]
            sumsq = ps[:, 512:512 + n]
            for ko in range(DC):
                nc.tensor.matmul(sums, onesPP, xb[:, ko, :n],
                                 start=(ko == 0), stop=(ko == DC - 1))
            for ko in range(DC):
                nc.tensor.matmul(sumsq, onesPP, xsq[:, ko, :n],
                                 start=(ko == 0), stop=(ko == DC - 1))
            mu_t = mupool.tile([128, 512], f32, tag="mu")
            mu = mu_t[:, :n]
            nc.vector.tensor_scalar_mul(mu, sums, 1.0 / D)
            t_t = lnpool.tile([128, 512], f32, tag="lntmp")
            t = t_t[:, :n]
            nc.vector.tensor_mul(t, mu, sums)
            nc.vector.tensor_tensor(t, sumsq, t, OP.subtract)
            nc.scalar.activation(t, t, AF.Sqrt, bias=epsT, scale=1.0 / D)
            rstd_t = mupool.tile([128, 512], f32, tag="rstd", bufs=1)
            rstd = rstd_t[:, :n]
            nc.vector.reciprocal_approx_fast(out=rstd, in_=t)
            return mu, rstd

        def ln_apply(out_tile):
            """out_tile[:, ko, t] = (x - mu) * rstd  (affine folded into weights)"""
            for c0, n in CHUNKS:
                sl = slice(c0, c0 + n)
                mu, rstd = ln_stats(c0, n)
                t = lnbig.tile([128, DC, 512], bf16, tag="lnsub")
                nc.vector.tensor_tensor(
                    t[:, :, :n], xT[:, :, sl],
                    mu[:, None, :].to_broadcast((128, DC, n)), OP.subtract)
                nc.vector.tensor_tensor(
                    out_tile[:, :, sl], t[:, :, :n],
                    rstd[:, None, :].to_broadcast((128, DC, n)), OP.mult)

        def ln_final(sT, bT):
            """final LN with affine, normalized part in bf16, in-place on xT."""
            for c0, n in CHUNKS:
                sl = slice(c0, c0 + n)
                mu, rstd = ln_stats(c0, n)
                t = lnbig.tile([128, DC, 512], bf16, tag="lnsub")
                nc.vector.tensor_tensor(
                    t[:, :, :n], xT[:, :, sl],
                    mu[:, None, :].to_broadcast((128, DC, n)), OP.subtract)
                nc.vector.tensor_tensor(
                    t[:, :, :n], t[:, :, :n],
                    rstd[:, None, :].to_broadcast((128, DC, n)), OP.mult)
                for ko in range(DC):
                    nc.vector.tensor_scalar(
                        out=xT[:, ko, sl], in0=t[:, ko, :n],
                        scalar1=sT[:, ko:ko + 1], scalar2=bT[:, ko:ko + 1],
                        op0=OP.mult, op1=OP.add)

        for l in range(n_layers):
            # ---------- prefetch this layer's weights ----------
            wqk_ch = dchunked(wqk_d[l])
            wq_tiles = []
            for i in range(3):
                wt = wpool.tile([128, DC, 512], f8, tag="w512", bufs=3)
                nc.sync.dma_start(wt, wqk_ch[:, :, 512 * i:512 * (i + 1)])
                wq_tiles.append(wt)
            wv8 = wpool.tile([128, DC, 768], f8, tag="wv8", bufs=1)
            nc.sync.dma_start(wv8, dchunked(wv_d[l]))
            wo8 = wpool.tile([128, DC, 768], f8, tag="wo8", bufs=1)
            nc.sync.dma_start(wo8, dchunked(wo_d[l]))
            bqkv = bpool.tile([128, 12], f32, tag="bqkv")
            nc.sync.dma_start(bqkv, bqkvT_d[l])
            b1T = bpool.tile([128, 24], f32, tag="b1T")
            nc.sync.dma_start(b1T, b1T_d[l])
            rbias = bpool.tile([1, 2, D], bf16, tag="rbias", bufs=1)
            nc.sync.dma_start(rbias, rbias_d[l])

            # ---------- LN1 (affine folded into wqkv/bqkv) ----------
            ln_apply(yT8)

            # ---------- V: natural layout -> vone slots ----------
            # (V bias folded host-side into the wo residual bias)
            for tt in range(NT):
                pv = psA_tile()
                for (cc0, cl) in ((0, 512), (512, 256)):
                    for j in range(3):
                        nc.tensor.matmul(
                            pv[:, cc0:cc0 + cl],
                            yT8[:, 2 * j:2 * j + 2, tt * 128:(tt + 1) * 128],
                            wv8[:, 2 * j:2 * j + 2, cc0:cc0 + cl],
                            start=(j == 0), stop=(j == 2), perf_mode=DR)
                vslots = vone[:, tt, :].rearrange("p (h s) -> p h s", s=128)
                nc.vector.tensor_scalar_mul(
                    vslots[:, 0:12, 0:64],
                    pv[:, 0:768].rearrange("p (h s) -> p h s", s=64), IWS)

            # ---------- QK per head pair + attention ----------
            OT = bigpool.tile([128, DC, TQ], f8, tag="OT")
            for pair in range(6):
                qk = qkpool.tile([128, 2, TP], bf16, tag="qk")
                nc.vector.memset(qk[:, :, TQ:TP], 0.0)     # dead-token K stays finite
                for c0, n in CHUNKS:
                    pq = psA_tile()
                    for i, m in enumerate((pair, 6 + pair)):
                        wt = wq_tiles[(m * 128) // 512]
                        coff = (m * 128) % 512
                        for j in range(3):
                            nc.tensor.matmul(pq[:, 512 * i:512 * i + n],
                                             wt[:, 2 * j:2 * j + 2, coff:coff + 128],
                                             yT8[:, 2 * j:2 * j + 2, c0:c0 + n],
                                             start=(j == 0), stop=(j == 2),
                                             perf_mode=DR)
                    for i, m in enumerate((pair, 6 + pair)):
                        nc.vector.tensor_scalar(
                            out=qk[:, i, c0:c0 + n], in0=pq[:, 512 * i:512 * i + n],
                            scalar1=IWS, scalar2=bqkv[:, m:m + 1],
                            op0=OP.mult, op1=OP.add)
                # scores for both heads of the pair: the two 64-contraction
                # matmuls land on disjoint PE row-groups and run concurrently;
                # one exp covers both heads via the paired PSUM tile.
                for kt in range(NT):
                    qs = 0 if kt == 0 else 128 * kt
                    off = ARENA_OFF[kt]
                    for g0 in range(qs, TQ, 512):
                        g1 = min(g0 + 512, TQ)
                        st = psA_tile()
                        for e in range(2):
                            nc.tensor.matmul(
                                st[:, 512 * e:512 * e + g1 - g0],
                                qk[64 * e:64 * e + 64, 1, kt * 128:(kt + 1) * 128],
                                qk[64 * e:64 * e + 64, 0, g0:g1],
                                start=True, stop=True)
                        nc.scalar.activation(
                            out=ptarena2[:, :, off + g0 - qs:off + g1 - qs],
                            in_=st[:].rearrange("p (e q) -> p e q", e=2)[:, :, 0:g1 - g0],
                            func=AF.Exp, bias=maskcol[:, kt:kt + 1], scale=0.125)
                    if 2 <= kt <= 10:
                        u = kt - 1
                        nc.vector.memset(
                            ptarena2[:, :, off + 1408 - qs:off + 1408 - qs + 8 * u], 0.0)
                    if kt == 11:
                        nc.vector.tensor_mul(
                            ptarena2[:, :, off:off + 80], ptarena2[:, :, off:off + 80],
                            rrm[:, None, 0:80].to_broadcast((128, 2, 80)))
                for e in range(2):
                    h = 2 * pair + e
                    for c0, n in CHUNKS:
                        c1_ = c0 + n
                        kts = [kt for kt in range(NT)
                               if (0 if kt == 0 else 128 * kt) < c1_]
                        ot = psB_tile()
                        for i, kt in enumerate(kts):
                            qs = 0 if kt == 0 else 128 * kt
                            off = ARENA_OFF[kt]
                            lo = max(qs, c0)
                            nc.tensor.matmul(ot[:, lo - c0:n],
                                             vone[:, kt, 128 * h:128 * h + 128],
                                             ptarena2[:, e, off + lo - qs:off + c1_ - qs],
                                             start=(i == 0), stop=(i == len(kts) - 1),
                                             skip_group_check=True)
                        rec = recpool.tile([128, 1024], f32, tag="rec")
                        nc.vector.tensor_copy(rec[0:64, 512:512 + n], ot[64:128, :n])
                        nc.vector.reciprocal_approx_fast(
                            out=rec[0:64, 0:n], in_=rec[0:64, 512:512 + n])
                        nc.vector.tensor_tensor(OT[64 * e:64 * e + 64, pair, c0:c1_],
                                                ot[0:64, :n], rec[0:64, 0:n], OP.mult)

            # ---------- WO + residual ----------
            for c0, n in CHUNKS:
                for dp in range(3):
                    pw = psA_tile()
                    for h2 in range(2):
                        dc = 2 * dp + h2
                        for j in range(3):
                            nc.tensor.matmul(pw[:, 512 * h2:512 * h2 + n],
                                             wo8[:, 2 * j:2 * j + 2, dc * 128:dc * 128 + 128],
                                             OT[:, 2 * j:2 * j + 2, c0:c0 + n],
                                             start=(j == 0), stop=False, perf_mode=DR)
                        nc.tensor.matmul(pw[:, 512 * h2:512 * h2 + n],
                                         rbias[0:1, 0, dc * 128:dc * 128 + 128],
                                         onerow[0:1, :n], start=False, stop=True)
                    for h2 in range(2):
                        dc = 2 * dp + h2
                        nc.vector.scalar_tensor_tensor(
                            out=xT[:, dc, c0:c0 + n],
                            in0=pw[:, 512 * h2:512 * h2 + n], scalar=IWS,
                            in1=xT[:, dc, c0:c0 + n],
                            op0=OP.mult, op1=OP.add)

            # ---------- FFN (bf16): LN2 per chunk, 4 F-quarters accumulated in PSUM ----------
            w1_ch = dchunked(w1_d[l])
            w2_ch = w2_d[l].rearrange("(fo p) n -> p fo n", p=128)
            for c0, n in CHUNKS:
                sl = slice(c0, c0 + n)
                mu, rstd = ln_stats(c0, n)
                y2c = y2pool.tile([128, DC, 512], bf16, tag="y2c")
                t2 = lnbig.tile([128, DC, 512], bf16, tag="lnsub")
                nc.vector.tensor_tensor(
                    t2[:, :, :n], xT[:, :, sl],
                    mu[:, None, :].to_broadcast((128, DC, n)), OP.subtract)
                nc.vector.tensor_tensor(
                    y2c[:, :, :n], t2[:, :, :n],
                    rstd[:, None, :].to_broadcast((128, DC, n)), OP.mult)
                accs = [psA_tile() for _ in range(3)]
                for q4 in range(4):
                    w1q = w1pool.tile([128, DC, 768], bf16, tag="w1q")
                    nc.sync.dma_start(w1q, w1_ch[:, :, 768 * q4:768 * q4 + 768])
                    w2q = w2pool.tile([128, DC, 768], bf16, tag="w2q", bufs=1)
                    nc.sync.dma_start(w2q, w2_ch[:, 6 * q4:6 * q4 + 6, :])
                    hT = hpool.tile([128, DC, 512], bf16, tag="hT")
                    for fm in range(6):
                        ph = psB_tile()
                        for ko in range(DC):
                            nc.tensor.matmul(ph[:, :n],
                                             w1q[:, ko, 128 * fm:128 * fm + 128],
                                             y2c[:, ko, :n],
                                             start=(ko == 0), stop=(ko == 5),
                                             skip_group_check=True)
                        nc.scalar.activation(out=hT[:, fm, :n], in_=ph[:, :n],
                                             func=AF.Gelu_apprx_tanh,
                                             bias=b1T[:, 6 * q4 + fm:6 * q4 + fm + 1])
                    for dc in range(6):
                        acc = accs[dc // 2][:, 512 * (dc % 2):512 * (dc % 2) + n]
                        for fo in range(6):
                            nc.tensor.matmul(acc, w2q[:, fo, 128 * dc:128 * dc + 128],
                                             hT[:, fo, :n],
                                             start=(q4 == 0 and fo == 0), stop=False)
                for dc in range(6):
                    acc = accs[dc // 2][:, 512 * (dc % 2):512 * (dc % 2) + n]
                    nc.tensor.matmul(acc, rbias[0:1, 1, dc * 128:dc * 128 + 128],
                                     onerow[0:1, :n], start=False, stop=True)
                    nc.vector.tensor_tensor(xT[:, dc, sl], xT[:, dc, sl], acc, OP.add)

        # ---------- final LN (in place) + store ----------
        ln_final(lnfs, lnfb)
        nc.sync.dma_start(dchunked(outT_d[:]), xT)

    nc.compile()
    return nc


# ---------------- host-side glue ----------------

def _f8(a):
    return np.clip(np.asarray(a, np.float32) * WS, -240.0, 240.0).astype(
        ml_dtypes.float8_e4m3)


def _prep_weights(inputs, n_layers):
    bf = ml_dtypes.bfloat16
    sl = slice(0, n_layers)

    def dT(a):  # [..., 768] -> [..., 128, 6] (d = ko*128 + p)
        return np.ascontiguousarray(np.swapaxes(a.reshape(*a.shape[:-1], DC, 128), -1, -2))

    ln1s = np.asarray(inputs["ln1_s"][sl], np.float32)
    ln1b = np.asarray(inputs["ln1_b"][sl], np.float32)
    ln2s = np.asarray(inputs["ln2_s"][sl], np.float32)
    ln2b = np.asarray(inputs["ln2_b"][sl], np.float32)

    # fold LN1 affine into wqkv/bqkv, LN2 affine into w1/b1
    wqkv = np.asarray(inputs["wqkv"][sl], np.float32)
    bqkv = np.asarray(inputs["bqkv"][sl], np.float32)
    wqkv_f = ln1s[:, :, None] * wqkv
    bqkv_f = bqkv + np.einsum('ld,lde->le', ln1b, wqkv)
    w1 = np.asarray(inputs["w1"][sl], np.float32)
    b1 = np.asarray(inputs["b1"][sl], np.float32)
    w1_f = ln2s[:, :, None] * w1
    b1_f = b1 + np.einsum('ld,lde->le', ln2b, w1)

    bqkvT = np.ascontiguousarray(
        np.swapaxes(bqkv_f[:, :1536].reshape(n_layers, 12, 128), 1, 2))
    bv = bqkv_f[:, 1536:]                        # V bias -> fold into wo bias
    b1T = np.ascontiguousarray(np.swapaxes(b1_f.reshape(n_layers, 24, 128), 1, 2))
    wo = np.asarray(inputs["wo"][sl], np.float32)
    bo_f = np.asarray(inputs["bo"][sl], np.float32) + np.einsum('ld,lde->le', bv, wo)
    b2 = np.asarray(inputs["b2"][sl], np.float32)
    rbias = np.stack([bo_f * WS, b2], axis=1).reshape(n_layers, 1, 2, D)
    return {
        "wqk": _f8(wqkv_f[:, :, :1536]),
        "wv": _f8(wqkv_f[:, :, 1536:]),
        "bqkvT": np.ascontiguousarray(bqkvT, np.float32),
        "wo": _f8(wo),
        "w1": np.ascontiguousarray(w1_f).astype(bf),
        "b1T": np.ascontiguousarray(b1T, np.float32),
        "w2": np.ascontiguousarray(np.asarray(inputs["w2"][sl], np.float32)).astype(bf),
        "rbias": np.ascontiguousarray(rbias).astype(bf),
        "lnfsT": np.ascontiguousarray(dT(np.asarray(inputs["lnf_s"], np.float32))),
        "lnfbT": np.ascontiguousarray(dT(np.asarray(inputs["lnf_b"], np.float32))),
    }


def _rrmask():
    m = np.zeros((128, 128), np.float32)
    for k in range(80):
        m[k, (k // 8) * 8:] = 1.0
    return m.astype(ml_dtypes.bfloat16)


def _maskcol(prefix_mask, obs_mask, readout_mask):
    """[128, 12] additive exp-bias per (key partition, key tile)."""
    m = np.full((128, NT), NEG, np.float32)
    m[:PFX, 0] = np.where(prefix_mask, 0.0, NEG)
    for t in range(HOR):
        m[:, 1 + t] = np.where(obs_mask[t], 0.0, NEG)
    ro = np.asarray(readout_mask).reshape(-1)
    m[:80, 11] = np.where(ro, 0.0, NEG)
    return m


def _assemble_xT(prefix, obs, readout):
    """(16,768),(10,128,768),(10,8,768) -> transposed padded (768,1536) f32"""
    x = np.zeros((TP, D), np.float32)
    x[:PFX] = prefix
    x[128:128 + HOR * 128] = obs.reshape(HOR * 128, D)
    x[1408:1408 + HOR * NR] = readout.reshape(HOR * NR, D)
    return np.ascontiguousarray(x.T)


def _gather_out(outT):
    """(768,1536) -> (1376, 768) in original token order"""
    xo = outT.T
    out = np.empty((T, D), np.float32)
    out[:PFX] = xo[:PFX]
    for t in range(HOR):
        out[PFX + TPS * t:PFX + TPS * t + NO] = xo[128 * (1 + t):128 * (2 + t)]
        out[PFX + TPS * t + NO:PFX + TPS * (t + 1)] = xo[1408 + NR * t:1408 + NR * (t + 1)]
    return out


def run(inputs, n_layers=L, trace=False, tmpdir=None):
    from concourse.bass_utils import run_bass_kernel_spmd

    key = ("nc", n_layers)
    if key not in _CACHE:
        _CACHE[key] = _build_nc(n_layers)
    nc = _CACHE[key]

    wmap = _prep_weights(inputs, n_layers)
    rr = _rrmask()
    pm = np.asarray(inputs["prefix_mask"], bool)
    om = np.asarray(inputs["obs_mask"], bool)
    rm = np.asarray(inputs["readout_mask"], bool)
    pt = np.asarray(inputs["prefix_tokens"], np.float32)
    ot = np.asarray(inputs["obs_tokens"], np.float32)
    rt = np.asarray(inputs["readout_tokens"], np.float32)

    in_maps = []
    for b in range(B):
        m = dict(wmap)
        m["xT"] = _assemble_xT(pt[b], ot[b], rt[b])
        m["maskcol"] = _maskcol(pm[b], om[b], rm[b])
        m["rrmask"] = rr
        in_maps.append(m)

    res = run_bass_kernel_spmd(nc, in_maps, list(range(B)), trace=trace, tmpdir=tmpdir)
    out = np.stack([_gather_out(res.results[b]["outT"]) for b in range(B)])
    return out, res


def kernel(**inputs):
    out, _ = run(inputs, L)
    return out


# revision 28
# speedup vs baseline: 1.1037x; 1.1037x over previous
"""Trainium2 Bass kernel for nn_BlockTransformer (Octo-style block-sparse transformer).

Strategy: data-parallel over batch (B=8 -> 1 element per NeuronCore), weights
replicated. Residual stream kept transposed (D on partitions). Tokens reordered
to [prefix|pad, obs t0..t9, readouts|pad] = 1536 padded tokens; per-key mask
folded into the softmax-exp bias, readout causality via small memsets + one
static multiplier tile.

v5: attention projections (Q/K, V, WO) in fp8e4m3 DoubleRow (256-deep
contraction, 2x bf16 rate on HW); weights scaled x256 host-side, compensated in
the following DVE op. Softmax's error-washing makes attention fp8 nearly free
accuracy-wise; the FFN stays bf16 (each fp8 tensor there costs ~1.3e-2 rel
err). One PSUM pool of 4x[128,1024] tiles (all 8 banks): scores pair the two
heads into halves so ONE exp covers both (half the ACT instructions), LN packs
sum|sumsq, QK packs the two weight columns, FFN packs fm/dc pairs. The
attention loop is software-pipelined: QK of pair p+1 issues before A@V of pair
p so the tensor queue has work while exp(p) drains on ACT. LN's bf16 staging
copy runs on the Scalar engine (GpSimd CAST is 3x slower and serialized LN).
WO/W2 biases enter via a 1-row matmul accumulated into PSUM.
"""
import sys
sys.path.insert(0, "/opt/trn_rl_repo")

import numpy as np
import ml_dtypes

B, HOR, PFX, NO, NR = 8, 10, 16, 128, 8
D, NH, HD, F, L = 768, 12, 64, 3072, 12
TPS = NO + NR
T = PFX + HOR * TPS          # 1376
TP = 1536                    # padded tokens (12 tiles of 128)
TQ = 1488                    # live tokens (prefix tile + 10 obs tiles + 80 readouts)
NT = TP // 128               # 12 token tiles
DC = D // 128                # 6 d-chunks
CHUNKS = [(0, 512), (512, 512), (1024, 464)]   # token chunks (c2 trimmed)
EPS = 1e-6
NEG = -30000.0
WS = 256.0                   # fp8 weight scale
IWS = 1.0 / WS

_CACHE = {}


def _build_nc(n_layers):
    from concourse import bacc
    import concourse.bass as bass
    import concourse.mybir as mybir
    import concourse.tile as tile
    from contextlib import ExitStack

    bf16, f32, f8 = mybir.dt.bfloat16, mybir.dt.float32, mybir.dt.float8e4
    AF = mybir.ActivationFunctionType
    OP = mybir.AluOpType
    DR = mybir.MatmulPerfMode.DoubleRow

    nc = bacc.Bacc("TRN2", num_devices=8)

    xT_d = nc.dram_tensor("xT", [D, TP], f32, kind="ExternalInput")
    wqk_d = nc.dram_tensor("wqk", [n_layers, D, 2 * D], f8, kind="ExternalInput")
    wv_d = nc.dram_tensor("wv", [n_layers, D, D], f8, kind="ExternalInput")
    bqkvT_d = nc.dram_tensor("bqkvT", [n_layers, 128, 12], f32, kind="ExternalInput")
    wo_d = nc.dram_tensor("wo", [n_layers, D, D], f8, kind="ExternalInput")
    w1_d = nc.dram_tensor("w1", [n_layers, D, F], bf16, kind="ExternalInput")
    b1T_d = nc.dram_tensor("b1T", [n_layers, 128, 24], f32, kind="ExternalInput")
    w2_d = nc.dram_tensor("w2", [n_layers, F, D], bf16, kind="ExternalInput")
    rbias_d = nc.dram_tensor("rbias", [n_layers, 1, 2, D], bf16, kind="ExternalInput")
    lnfsT_d = nc.dram_tensor("lnfsT", [128, DC], f32, kind="ExternalInput")
    lnfbT_d = nc.dram_tensor("lnfbT", [128, DC], f32, kind="ExternalInput")
    maskcol_d = nc.dram_tensor("maskcol", [128, NT], f32, kind="ExternalInput")
    rrmask_d = nc.dram_tensor("rrmask", [128, 128], bf16, kind="ExternalInput")
    outT_d = nc.dram_tensor("outT", [D, TP], f32, kind="ExternalOutput")

    def dchunked(ap):  # [D, N] dram AP -> [128, DC-chunks, N]
        return ap.rearrange("(ko p) n -> p ko n", p=128)

    with tile.TileContext(nc) as tc, ExitStack() as ctx:
        const = ctx.enter_context(tc.tile_pool(name="const", bufs=1))
        persist = ctx.enter_context(tc.tile_pool(name="persist", bufs=1))
        wpool = ctx.enter_context(tc.tile_pool(name="wpool", bufs=1))
        bpool = ctx.enter_context(tc.tile_pool(name="bpool", bufs=2))
        bigpool = ctx.enter_context(tc.tile_pool(name="bigpool", bufs=1))
        qkpool = ctx.enter_context(tc.tile_pool(name="qkpool", bufs=2))
        lnpool = ctx.enter_context(tc.tile_pool(name="lnpool", bufs=1))
        mupool = ctx.enter_context(tc.tile_pool(name="mupool", bufs=2))
        lnbig = ctx.enter_context(tc.tile_pool(name="lnbig", bufs=1))
        recpool = ctx.enter_context(tc.tile_pool(name="recpool", bufs=1))
        psA = ctx.enter_context(tc.tile_pool(name="psA", bufs=4, space="PSUM"))

        def psA_tile():
            return psA.tile([128, 1024], f32, tag="a", name="psa")

        # ---- persistent state ----
        xT = persist.tile([128, DC, TP], f32)         # residual stream (transposed)
        nc.sync.dma_start(xT, dchunked(xT_d[:]))
        vone = persist.tile([128, NT, NH * 128], f8)  # per head: [V_h | ones]
        nc.vector.memset(vone, 1.0)
        yT8 = persist.tile([128, DC, TP], f8)         # LN1 out (shared by QKV)
        nc.vector.memset(yT8[:, :, TQ:TP], 0.0)
        yT2 = persist.tile([128, DC, TQ], bf16)       # LN2 out (FFN input)
        ARENA_OFF = []
        _o = 0
        for kt in range(NT):
            ARENA_OFF.append(_o)
            _o += TQ - (0 if kt == 0 else 128 * kt)
        # two fp8 exp(score) arenas -> both heads of a pair live concurrently
        ptarena2 = persist.tile([128, 2, _o], f8, tag="ptarena2")

        # ---- constants ----
        maskcol = const.tile([128, NT], f32)
        nc.sync.dma_start(maskcol, maskcol_d[:])
        rrm = const.tile([128, 128], bf16)
        nc.sync.dma_start(rrm, rrmask_d[:])
        onesPP = const.tile([128, 128], bf16)
        nc.vector.memset(onesPP, 1.0)
        onerow = const.tile([1, 512], bf16)
        nc.vector.memset(onerow, 1.0)
        epsT = const.tile([128, 1], f32)
        nc.vector.memset(epsT, EPS)
        lnfs = const.tile([128, DC], f32)
        nc.sync.dma_start(lnfs, lnfsT_d[:])
        lnfb = const.tile([128, DC], f32)
        nc.sync.dma_start(lnfb, lnfbT_d[:])

        def ln_stats(c0, n):
            """returns (mu, rstd), both f32 [128, n], for token chunk [c0, c0+n)."""
            sl = slice(c0, c0 + n)
            xsq = lnbig.tile([128, DC, 512], bf16, tag="xsq")
            nc.scalar.activation(xsq[:, :, :n], xT[:, :, sl], AF.Square)
            xb = lnbig.tile([128, DC, 512], bf16, tag="xb")
            nc.scalar.activation(xb[:, :, :n], xT[:, :, sl], AF.Copy)
            ps = psA_tile()
            sums = ps[:, :n]
            sumsq = ps[:, 512:512 + n]
            for ko in range(DC):
                nc.tensor.matmul(sums, onesPP, xb[:, ko, :n],
                                 start=(ko == 0), stop=(ko == DC - 1))
            for ko in range(DC):
                nc.tensor.matmul(sumsq, onesPP, xsq[:, ko, :n],
                                 start=(ko == 0), stop=(ko == DC - 1))
            mu_t = mupool.tile([128, 512], f32, tag="mu", bufs=1)
            mu = mu_t[:, :n]
            nc.vector.tensor_scalar_mul(mu, sums, 1.0 / D)
            t_t = lnpool.tile([128, 512], f32, tag="lntmp")
            t = t_t[:, :n]
            nc.vector.tensor_mul(t, mu, sums)
            nc.vector.tensor_tensor(t, sumsq, t, OP.subtract)
            nc.scalar.activation(t, t, AF.Sqrt, bias=epsT, scale=1.0 / D)
            rstd_t = mupool.tile([128, 512], f32, tag="rstd", bufs=1)
            rstd = rstd_t[:, :n]
            nc.vector.reciprocal_approx_fast(out=rstd, in_=t)
            return mu, rstd

        def ln_apply(out_tile):
            """out_tile[:, ko, t] = (x - mu) * rstd  (affine folded into weights)"""
            for c0, n in CHUNKS:
                sl = slice(c0, c0 + n)
                mu, rstd = ln_stats(c0, n)
                t = lnbig.tile([128, DC, 512], bf16, tag="lnsub")
                nc.vector.tensor_tensor(
                    t[:, :, :n], xT[:, :, sl],
                    mu[:, None, :].to_broadcast((128, DC, n)), OP.subtract)
                nc.vector.tensor_tensor(
                    out_tile[:, :, sl], t[:, :, :n],
                    rstd[:, None, :].to_broadcast((128, DC, n)), OP.mult)

        def ln_final(sT, bT):
            """final LN with affine, in-place on xT."""
            for c0, n in CHUNKS:
                sl = slice(c0, c0 + n)
                mu, rstd = ln_stats(c0, n)
                t = lnbig.tile([128, DC, 512], bf16, tag="lnsub")
                nc.vector.tensor_tensor(
                    t[:, :, :n], xT[:, :, sl],
                    mu[:, None, :].to_broadcast((128, DC, n)), OP.subtract)
                nc.vector.tensor_tensor(
                    t[:, :, :n], t[:, :, :n],
                    rstd[:, None, :].to_broadcast((128, DC, n)), OP.mult)
                for ko in range(DC):
                    nc.vector.tensor_scalar(
                        out=xT[:, ko, sl], in0=t[:, ko, :n],
                        scalar1=sT[:, ko:ko + 1], scalar2=bT[:, ko:ko + 1],
                        op0=OP.mult, op1=OP.add)

        for l in range(n_layers):
            # ---------- prefetch attention weights ----------
            wqk_ch = dchunked(wqk_d[l])
            wq_tiles = []
            for i in range(3):
                wt = wpool.tile([128, DC, 512], f8, tag="w512", bufs=3)
                nc.sync.dma_start(wt, wqk_ch[:, :, 512 * i:512 * (i + 1)])
                wq_tiles.append(wt)
            wv8 = wpool.tile([128, DC, 768], f8, tag="wv8", bufs=1)
            nc.sync.dma_start(wv8, dchunked(wv_d[l]))
            wo8 = wpool.tile([128, DC, 768], f8, tag="wo8", bufs=1)
            nc.sync.dma_start(wo8, dchunked(wo_d[l]))
            bqkv = bpool.tile([128, 12], f32, tag="bqkv")
            nc.sync.dma_start(bqkv, bqkvT_d[l])
            b1T = bpool.tile([128, 24], f32, tag="b1T")
            nc.sync.dma_start(b1T, b1T_d[l])
            rbias = bpool.tile([1, 2, D], bf16, tag="rbias", bufs=1)
            nc.sync.dma_start(rbias, rbias_d[l])

            # ---------- LN1 (affine folded into wqkv/bqkv) ----------
            ln_apply(yT8)

            # ---------- V: natural layout -> vone slots ----------
            # (V bias folded host-side into the wo residual bias)
            for tt in range(NT):
                pv = psA_tile()
                for (cc0, cl) in ((0, 512), (512, 256)):
                    for j in range(3):
                        nc.tensor.matmul(
                            pv[:, cc0:cc0 + cl],
                            yT8[:, 2 * j:2 * j + 2, tt * 128:(tt + 1) * 128],
                            wv8[:, 2 * j:2 * j + 2, cc0:cc0 + cl],
                            start=(j == 0), stop=(j == 2), perf_mode=DR)
                vslots = vone[:, tt, :].rearrange("p (h s) -> p h s", s=128)
                nc.vector.tensor_scalar_mul(
                    vslots[:, 0:12, 0:64],
                    pv[:, 0:768].rearrange("p (h s) -> p h s", s=64), IWS)

            # ---------- attention: QK(p+1) issues before A@V(p) ----------
            def compute_qk(pair):
                qk = qkpool.tile([128, 2, TP], bf16, tag="qk", name="qk")
                nc.vector.memset(qk[:, :, TQ:TP], 0.0)   # dead-token K stays finite
                for c0, n in CHUNKS:
                    pq = psA_tile()
                    for i, m in enumerate((pair, 6 + pair)):
                        wt = wq_tiles[(m * 128) // 512]
                        coff = (m * 128) % 512
                        for j in range(3):
                            nc.tensor.matmul(pq[:, 512 * i:512 * i + n],
                                             wt[:, 2 * j:2 * j + 2, coff:coff + 128],
                                             yT8[:, 2 * j:2 * j + 2, c0:c0 + n],
                                             start=(j == 0), stop=(j == 2),
                                             perf_mode=DR)
                    for i, m in enumerate((pair, 6 + pair)):
                        nc.vector.tensor_scalar(
                            out=qk[:, i, c0:c0 + n], in0=pq[:, 512 * i:512 * i + n],
                            scalar1=IWS, scalar2=bqkv[:, m:m + 1],
                            op0=OP.mult, op1=OP.add)
                return qk

            def scores(qk):
                # the two 64-contraction matmuls land on disjoint PE row-groups
                # and run concurrently; one exp covers both heads via the
                # paired PSUM tile.
                for kt in range(NT):
                    qs = 0 if kt == 0 else 128 * kt
                    off = ARENA_OFF[kt]
                    for g0 in range(qs, TQ, 512):
                        g1 = min(g0 + 512, TQ)
                        st = psA_tile()
                        for e in range(2):
                            nc.tensor.matmul(
                                st[:, 512 * e:512 * e + g1 - g0],
                                qk[64 * e:64 * e + 64, 1, kt * 128:(kt + 1) * 128],
                                qk[64 * e:64 * e + 64, 0, g0:g1],
                                start=True, stop=True)
                        nc.scalar.activation(
                            out=ptarena2[:, :, off + g0 - qs:off + g1 - qs],
                            in_=st[:].rearrange("p (e q) -> p e q", e=2)[:, :, 0:g1 - g0],
                            func=AF.Exp, bias=maskcol[:, kt:kt + 1], scale=0.125)
                    if 2 <= kt <= 10:
                        u = kt - 1
                        nc.vector.memset(
                            ptarena2[:, :, off + 1408 - qs:off + 1408 - qs + 8 * u], 0.0)
                    if kt == 11:
                        nc.vector.tensor_mul(
                            ptarena2[:, :, off:off + 80], ptarena2[:, :, off:off + 80],
                            rrm[:, None, 0:80].to_broadcast((128, 2, 80)))

            def att_v(pair, OT):
                for c0, n in CHUNKS:
                    c1_ = c0 + n
                    kts = [kt for kt in range(NT)
                           if (0 if kt == 0 else 128 * kt) < c1_]
                    ot2 = psA_tile()
                    for e in range(2):
                        h = 2 * pair + e
                        for i, kt in enumerate(kts):
                            qs = 0 if kt == 0 else 128 * kt
                            off = ARENA_OFF[kt]
                            lo = max(qs, c0)
                            nc.tensor.matmul(
                                ot2[:, 512 * e + lo - c0:512 * e + n],
                                vone[:, kt, 128 * h:128 * h + 128],
                                ptarena2[:, e, off + lo - qs:off + c1_ - qs],
                                start=(i == 0), stop=(i == len(kts) - 1),
                                skip_group_check=True)
                    rec = recpool.tile([128, 1024], f32, tag="rec")
                    for e in range(2):
                        nc.vector.tensor_copy(rec[0:64, 512 * e:512 * e + n],
                                              ot2[64:128, 512 * e:512 * e + n])
                        nc.vector.reciprocal_approx_fast(
                            out=rec[0:64, 512 * e:512 * e + n],
                            in_=rec[0:64, 512 * e:512 * e + n])
                        nc.vector.tensor_tensor(
                            OT[64 * e:64 * e + 64, pair, c0:c1_],
                            ot2[0:64, 512 * e:512 * e + n],
                            rec[0:64, 512 * e:512 * e + n], OP.mult)

            OT = bigpool.tile([128, DC, TQ], f8, tag="OT")
            qk_cur = compute_qk(0)
            for pair in range(6):
                scores(qk_cur)
                if pair < 5:
                    qk_cur = compute_qk(pair + 1)
                att_v(pair, OT)

            # ---------- WO + residual ----------
            for c0, n in CHUNKS:
                for dp in range(3):
                    pw = psA_tile()
                    for h2 in range(2):
                        dc = 2 * dp + h2
                        for j in range(3):
                            nc.tensor.matmul(pw[:, 512 * h2:512 * h2 + n],
                                             wo8[:, 2 * j:2 * j + 2, dc * 128:dc * 128 + 128],
                                             OT[:, 2 * j:2 * j + 2, c0:c0 + n],
                                             start=(j == 0), stop=False, perf_mode=DR)
                        nc.tensor.matmul(pw[:, 512 * h2:512 * h2 + n],
                                         rbias[0:1, 0, dc * 128:dc * 128 + 128],
                                         onerow[0:1, :n], start=False, stop=True)
                    for h2 in range(2):
                        dc = 2 * dp + h2
                        nc.vector.scalar_tensor_tensor(
                            out=xT[:, dc, c0:c0 + n],
                            in0=pw[:, 512 * h2:512 * h2 + n], scalar=IWS,
                            in1=xT[:, dc, c0:c0 + n],
                            op0=OP.mult, op1=OP.add)

            # ---------- LN2 (affine folded into w1/b1) ----------
            ln_apply(yT2)

            # ---------- FFN (bf16, 4 quarters of F) ----------
            w1_ch = dchunked(w1_d[l])
            w2_ch = w2_d[l].rearrange("(fo p) n -> p fo n", p=128)
            for q4 in range(4):
                f0 = 768 * q4
                w1_a = wpool.tile([128, DC, 512], bf16, tag="wffn", bufs=3)
                nc.sync.dma_start(w1_a, w1_ch[:, :, f0:f0 + 512])
                w1_b = wpool.tile([128, DC, 256], bf16, tag="wffn", bufs=3)
                nc.sync.dma_start(w1_b, w1_ch[:, :, f0 + 512:f0 + 768])
                hT = bigpool.tile([128, DC, TQ], bf16, tag="hT")
                for fp_ in range(3):
                    for c0, n in CHUNKS:
                        ph = psA_tile()
                        for half in range(2):
                            fm = 2 * fp_ + half
                            wt, coff = (w1_a, fm * 128) if fm < 4 else (w1_b, (fm - 4) * 128)
                            for ko in range(DC):
                                nc.tensor.matmul(ph[:, 512 * half:512 * half + n],
                                                 wt[:, ko, coff:coff + 128],
                                                 yT2[:, ko, c0:c0 + n],
                                                 start=(ko == 0), stop=(ko == DC - 1))
                        for half in range(2):
                            fm = 2 * fp_ + half
                            nc.scalar.activation(
                                out=hT[:, fm, c0:c0 + n],
                                in_=ph[:, 512 * half:512 * half + n],
                                func=AF.Gelu_apprx_tanh,
                                bias=b1T[:, 6 * q4 + fm:6 * q4 + fm + 1])
                w2_a = wpool.tile([128, DC, 512], bf16, tag="wffn", bufs=3)
                nc.sync.dma_start(w2_a, w2_ch[:, 6 * q4:6 * q4 + 6, 0:512])
                w2_b = wpool.tile([128, DC, 256], bf16, tag="wffn", bufs=3)
                nc.sync.dma_start(w2_b, w2_ch[:, 6 * q4:6 * q4 + 6, 512:768])
                for c0, n in CHUNKS:
                    for dp in range(3):
                        pw = psA_tile()
                        for half in range(2):
                            dc = 2 * dp + half
                            wt, coff = (w2_a, dc * 128) if dc < 4 else (w2_b, (dc - 4) * 128)
                            for fo in range(DC):
                                nc.tensor.matmul(pw[:, 512 * half:512 * half + n],
                                                 wt[:, fo, coff:coff + 128],
                                                 hT[:, fo, c0:c0 + n],
                                                 start=(fo == 0),
                                                 stop=(fo == DC - 1 and q4 > 0))
                            if q4 == 0:
                                nc.tensor.matmul(pw[:, 512 * half:512 * half + n],
                                                 rbias[0:1, 1, dc * 128:dc * 128 + 128],
                                                 onerow[0:1, :n], start=False, stop=True)
                        for half in range(2):
                            dc = 2 * dp + half
                            nc.vector.tensor_tensor(
                                xT[:, dc, c0:c0 + n], xT[:, dc, c0:c0 + n],
                                pw[:, 512 * half:512 * half + n], OP.add)

        # ---------- final LN (in place) + store ----------
        ln_final(lnfs, lnfb)
        nc.sync.dma_start(dchunked(outT_d[:]), xT)

    nc.compile()
    return nc


# ---------------- host-side glue ----------------

def _f8(a):
    return np.clip(np.asarray(a, np.float32) * WS, -240.0, 240.0).astype(
        ml_dtypes.float8_e4m3)


def _prep_weights(inputs, n_layers):
    bf = ml_dtypes.bfloat16
    sl = slice(0, n_layers)

    def dT(a):  # [..., 768] -> [..., 128, 6] (d = ko*128 + p)
        return np.ascontiguousarray(np.swapaxes(a.reshape(*a.shape[:-1], DC, 128), -1, -2))

    ln1s = np.asarray(inputs["ln1_s"][sl], np.float32)
    ln1b = np.asarray(inputs["ln1_b"][sl], np.float32)
    ln2s = np.asarray(inputs["ln2_s"][sl], np.float32)
    ln2b = np.asarray(inputs["ln2_b"][sl], np.float32)

    # fold LN1 affine into wqkv/bqkv, LN2 affine into w1/b1
    wqkv = np.asarray(inputs["wqkv"][sl], np.float32)
    bqkv = np.asarray(inputs["bqkv"][sl], np.float32)
    wqkv_f = ln1s[:, :, None] * wqkv
    bqkv_f = bqkv + np.einsum('ld,lde->le', ln1b, wqkv)
    w1 = np.asarray(inputs["w1"][sl], np.float32)
    b1 = np.asarray(inputs["b1"][sl], np.float32)
    w1_f = ln2s[:, :, None] * w1
    b1_f = b1 + np.einsum('ld,lde->le', ln2b, w1)

    bqkvT = np.ascontiguousarray(
        np.swapaxes(bqkv_f[:, :1536].reshape(n_layers, 12, 128), 1, 2))
    bv = bqkv_f[:, 1536:]                        # V bias -> fold into wo bias
    b1T = np.ascontiguousarray(np.swapaxes(b1_f.reshape(n_layers, 24, 128), 1, 2))
    wo = np.asarray(inputs["wo"][sl], np.float32)
    bo_f = np.asarray(inputs["bo"][sl], np.float32) + np.einsum('ld,lde->le', bv, wo)
    b2 = np.asarray(inputs["b2"][sl], np.float32)
    rbias = np.stack([bo_f * WS, b2], axis=1).reshape(n_layers, 1, 2, D)
    return {
        "wqk": _f8(wqkv_f[:, :, :1536]),
        "wv": _f8(wqkv_f[:, :, 1536:]),
        "bqkvT": np.ascontiguousarray(bqkvT, np.float32),
        "wo": _f8(wo),
        "w1": np.ascontiguousarray(w1_f).astype(bf),
        "b1T": np.ascontiguousarray(b1T, np.float32),
        "w2": np.ascontiguousarray(np.asarray(inputs["w2"][sl], np.float32)).astype(bf),
        "rbias": np.ascontiguousarray(rbias).astype(bf),
        "lnfsT": np.ascontiguousarray(dT(np.asarray(inputs["lnf_s"], np.float32))),
        "lnfbT": np.ascontiguousarray(dT(np.asarray(inputs["lnf_b"], np.float32))),
    }


def _rrmask():
    m = np.zeros((128, 128), np.float32)
    for k in range(80):
        m[k, (k // 8) * 8:] = 1.0
    return m.astype(ml_dtypes.bfloat16)


def _maskcol(prefix_mask, obs_mask, readout_mask):
    """[128, 12] additive exp-bias per (key partition, key tile)."""
    m = np.full((128, NT), NEG, np.float32)
    m[:PFX, 0] = np.where(prefix_mask, 0.0, NEG)
    for t in range(HOR):
        m[:, 1 + t] = np.where(obs_mask[t], 0.0, NEG)
    ro = np.asarray(readout_mask).reshape(-1)
    m[:80, 11] = np.where(ro, 0.0, NEG)
    return m


def _assemble_xT(prefix, obs, readout):
    """(16,768),(10,128,768),(10,8,768) -> transposed padded (768,1536) f32"""
    x = np.zeros((TP, D), np.float32)
    x[:PFX] = prefix
    x[128:128 + HOR * 128] = obs.reshape(HOR * 128, D)
    x[1408:1408 + HOR * NR] = readout.reshape(HOR * NR, D)
    return np.ascontiguousarray(x.T)


def _gather_out(outT):
    """(768,1536) -> (1376, 768) in original token order"""
    xo = outT.T
    out = np.empty((T, D), np.float32)
    out[:PFX] = xo[:PFX]
    for t in range(HOR):
        out[PFX + TPS * t:PFX + TPS * t + NO] = xo[128 * (1 + t):128 * (2 + t)]
        out[PFX + TPS * t + NO:PFX + TPS * (t + 1)] = xo[1408 + NR * t:1408 + NR * (t + 1)]
    return out


def run(inputs, n_layers=L, trace=False, tmpdir=None):
    from concourse.bass_utils import run_bass_kernel_spmd

    key = ("nc", n_layers)
    if key not in _CACHE:
        _CACHE[key] = _build_nc(n_layers)
    nc = _CACHE[key]

    wmap = _prep_weights(inputs, n_layers)
    rr = _rrmask()
    pm = np.asarray(inputs["prefix_mask"], bool)
    om = np.asarray(inputs["obs_mask"], bool)
    rm = np.asarray(inputs["readout_mask"], bool)
    pt = np.asarray(inputs["prefix_tokens"], np.float32)
    ot = np.asarray(inputs["obs_tokens"], np.float32)
    rt = np.asarray(inputs["readout_tokens"], np.float32)

    in_maps = []
    for b in range(B):
        m = dict(wmap)
        m["xT"] = _assemble_xT(pt[b], ot[b], rt[b])
        m["maskcol"] = _maskcol(pm[b], om[b], rm[b])
        m["rrmask"] = rr
        in_maps.append(m)

    res = run_bass_kernel_spmd(nc, in_maps, list(range(B)), trace=trace, tmpdir=tmpdir)
    out = np.stack([_gather_out(res.results[b]["outT"]) for b in range(B)])
    return out, res


def kernel(**inputs):
    out, _ = run(inputs, L)
    return out
